# revision 10
# baseline (speedup 1.0000x reference)
"""BEiT-style windowed attention (B=32, N=577, D=768, 12 heads) on 8 TRN2 cores.

Strategy: pure data-parallel over batch (4 batch elements per core, no
collectives). All matmuls in bf16 (fp32 PSUM accumulate), softmax in fp32.

Per-core dataflow (all activations kept SBUF-resident per batch element):
  qkT  [d,tok] = W_qkv(T) @ xT          (q/k produced transposed)
  v    [tok,d] = xT(T) @ W_qkvT          (v produced natural, +ones col)
  S.T  [k,q]   = kT(T) @ qT              (scores transposed, K=64)
  P    = exp(S.T + relbias.T)            (unsafe softmax: logits are tiny)
  O_un [q,d+1] = P(T) @ v_aug            (ones col gives rowsum per q)
  O    = O_un[:, :64] * recip(rowsum)    (per-partition scale)
  OT   = transpose(O)                    (PE transposes, 30 per batch)
  out  [tok,od] = OT(T) @ W_projT + b    (natural layout -> contiguous DMA)

Host-side prep: transposed/bf16 weights, x transposed to [d,tok] tiles,
relative-position bias table gathered + transposed, softmax scale folded
into the q rows of W_qkv.
"""

import numpy as np
import ml_dtypes

import concourse.bass as bass
import concourse.tile as tile
from concourse import bacc
from concourse import mybir
from concourse.bass_utils import run_bass_kernel_spmd
from concourse.masks import make_identity

B, N, D = 32, 577, 768
NH, DH = 12, 64
NCORES = 8
BL = B // NCORES            # 4 batch elements per core
SCALE = DH ** -0.5
KT = D // 128               # 6 contraction tiles over D
TT = (N + 127) // 128       # 5 token tiles (4x128 + 65)
BF16 = ml_dtypes.bfloat16

F32 = mybir.dt.float32
BF = mybir.dt.bfloat16


def tok_m(t):
    return min(128, N - 128 * t)


def _build_nc():
    nc = bacc.Bacc()

    xT_d = nc.declare_dram_parameter("xT", [BL, 128, KT, N], BF, isOutput=False)
    wqkv_d = nc.declare_dram_parameter("wqkv", [128, KT, 3 * D], BF, isOutput=False)
    wproj_d = nc.declare_dram_parameter("wproj", [128, KT, D], BF, isOutput=False)
    biasT_d = nc.declare_dram_parameter("biasT", [128, NH, TT, N], BF, isOutput=False)
    qkvb_d = nc.declare_dram_parameter("qkvb", [128, 18], F32, isOutput=False)
    vb_d = nc.declare_dram_parameter("vb", [1, D], F32, isOutput=False)
    pb_d = nc.declare_dram_parameter("pb", [1, D], F32, isOutput=False)
    out_d = nc.declare_dram_parameter("out", [BL, N, D], F32, isOutput=True)

    Exp = mybir.ActivationFunctionType.Exp
    QCH = [(0, 512), (512, N - 512)]          # free-dim chunks over 577
    DCH = [(0, 512), (512, D - 512)]          # free-dim chunks over 768

    with tile.TileContext(nc) as tc:
        with (
            tc.tile_pool(name="singles", bufs=1) as singles,
            tc.tile_pool(name="xt", bufs=2) as xt_pool,
            tc.tile_pool(name="qkt", bufs=1) as qkt_pool,
            tc.tile_pool(name="vbuf", bufs=1) as v_pool,
            tc.tile_pool(name="exps", bufs=2) as exps_pool,
            tc.tile_pool(name="sf32", bufs=2) as sf32_pool,
            tc.tile_pool(name="obuf", bufs=1) as o_pool,
            tc.tile_pool(name="otb", bufs=1) as ot_pool,
            tc.tile_pool(name="outs", bufs=2) as out_pool,
            tc.tile_pool(name="small", bufs=4) as small_pool,
            tc.tile_pool(name="ps512", bufs=4, space="PSUM") as ps512,
            tc.tile_pool(name="ps128", bufs=4, space="PSUM") as ps128,
        ):
            # ---- one-time loads ----
            w_qkv = singles.tile([128, KT, 3 * D], BF)
            nc.sync.dma_start(out=w_qkv, in_=wqkv_d[:])
            w_proj = singles.tile([128, KT, D], BF)
            nc.sync.dma_start(out=w_proj, in_=wproj_d[:])
            biasT = singles.tile([128, NH, TT, N], BF)
            nc.sync.dma_start(out=biasT, in_=biasT_d[:])
            qkvb = singles.tile([128, 18], F32)
            nc.sync.dma_start(out=qkvb, in_=qkvb_d[:])
            vbias = singles.tile([128, D], F32)
            nc.sync.dma_start(out=vbias, in_=vb_d[:].to_broadcast([128, D]))
            pbias = singles.tile([128, D], F32)
            nc.sync.dma_start(out=pbias, in_=pb_d[:].to_broadcast([128, D]))
            ident = singles.tile([128, 128], BF)
            make_identity(nc, ident)

            for b in range(BL):
                xT = xt_pool.tile([128, KT, N], BF)
                nc.sync.dma_start(out=xT, in_=xT_d[b])

                # ---- qkv projection: q,k transposed [d, tok] ----
                qkT = qkt_pool.tile([128, 2 * KT, N], BF)
                for mt in range(2 * KT):
                    pss = [ps512.tile([128, 512], F32, name="ps_qk0", tag="a"),
                           ps128.tile([128, 128], F32, name="ps_qk1", tag="b")]
                    for kt in range(KT):
                        for ci, (c0, w) in enumerate(QCH):
                            nc.tensor.matmul(
                                pss[ci][:, :w],
                                w_qkv[:, kt, 128 * mt:128 * (mt + 1)],
                                xT[:, kt, c0:c0 + w],
                                start=(kt == 0), stop=(kt == KT - 1),
                            )
                    for ci, (c0, w) in enumerate(QCH):
                        nc.vector.tensor_add(
                            qkT[:, mt, c0:c0 + w], pss[ci][:, :w],
                            qkvb[:, mt:mt + 1].to_broadcast([128, w]),
                        )

                # ---- qkv projection: v natural [tok, d], strided + ones col ----
                v_sb = v_pool.tile([128, TT, NH * 65], BF)
                v_str = v_sb.rearrange("p t (h c) -> p t h c", c=65)
                nc.vector.memset(v_str[:, :, :, 64:65], 1.0)
                for tt in range(TT):
                    m = tok_m(tt)
                    pss = [ps512.tile([128, 512], F32, name="ps_v0", tag="a"),
                           ps512.tile([128, 512], F32, name="ps_v1", tag="a")]
                    for kt in range(KT):
                        for ci, (c0, w) in enumerate(DCH):
                            nc.tensor.matmul(
                                pss[ci][:m, :w],
                                xT[:, kt, 128 * tt:128 * tt + m],
                                w_qkv[:, kt, 2 * D + c0:2 * D + c0 + w],
                                start=(kt == 0), stop=(kt == KT - 1),
                            )
                    for ci, (c0, w) in enumerate(DCH):
                        nh0, nh1 = c0 // 64, (c0 + w) // 64
                        nc.vector.tensor_add(
                            v_str[:m, tt, nh0:nh1, 0:64],
                            pss[ci][:m, :w].rearrange("p (h c) -> p h c", c=64),
                            vbias[:m, c0:c0 + w].rearrange("p (h c) -> p h c", c=64),
                        )

                # ---- attention per head ----
                o_sb = o_pool.tile([128, TT, D], BF)
                for h in range(NH):
                    qT = qkT[64 * (h % 2):64 * (h % 2) + 64, h // 2, :]
                    kTh = qkT[64 * (h % 2):64 * (h % 2) + 64, KT + h // 2, :]
                    expS = exps_pool.tile([128, TT, N], BF)
                    for kt in range(TT):
                        km = tok_m(kt)
                        sf = sf32_pool.tile([128, N], F32)
                        ps_s = [ps512.tile([128, 512], F32, name="ps_s0", tag="a"),
                                ps128.tile([128, 128], F32, name="ps_s1", tag="b")]
                        for ci, (c0, w) in enumerate(QCH):
                            nc.tensor.matmul(
                                ps_s[ci][:km, :w],
                                kTh[:, 128 * kt:128 * kt + km],
                                qT[:, c0:c0 + w],
                                start=True, stop=True,
                            )
                            nc.vector.tensor_add(
                                sf[:km, c0:c0 + w], ps_s[ci][:km, :w],
                                biasT[:km, h, kt, c0:c0 + w],
                            )
                        nc.scalar.activation(expS[:km, kt, :], sf[:km, :], Exp)
                    # P @ V_aug  (out natural [q, 64+1]; col 64 = rowsum)
                    for qt in range(TT):
                        qm = tok_m(qt)
                        ps_o = ps128.tile([128, 128], F32, name="ps_o", tag="b")
                        for kt in range(TT):
                            km = tok_m(kt)
                            nc.tensor.matmul(
                                ps_o[:qm, :65],
                                expS[:km, kt, 128 * qt:128 * qt + qm],
                                v_sb[:km, kt, 65 * h:65 * h + 65],
                                start=(kt == 0), stop=(kt == TT - 1),
                            )
                        rcp = small_pool.tile([128, 1], F32)
                        nc.vector.reciprocal(rcp[:qm], ps_o[:qm, 64:65])
                        nc.vector.tensor_mul(
                            o_sb[:qm, qt, 64 * h:64 * h + 64],
                            ps_o[:qm, 0:64],
                            rcp[:qm, 0:1].to_broadcast([qm, 64]),
                        )

                # ---- transpose O -> OT [d, tok] ----
                oT = ot_pool.tile([128, KT, N], BF)
                for qt in range(TT):
                    qm = tok_m(qt)
                    for dt in range(KT):
                        ps_t = ps128.tile([128, 128], BF, name="ps_t", tag="b")
                        nc.tensor.transpose(
                            ps_t[:, :qm],
                            o_sb[:qm, qt, 128 * dt:128 * (dt + 1)],
                            ident[:qm, :qm],
                        )
                        nc.scalar.activation(
                            oT[:, dt, 128 * qt:128 * qt + qm], ps_t[:, :qm],
                            mybir.ActivationFunctionType.Copy,
                        )

                # ---- output projection ----
                for tt in range(TT):
                    m = tok_m(tt)
                    out_sb = out_pool.tile([128, D], F32)
                    pss = [ps512.tile([128, 512], F32, name="ps_p0", tag="a"),
                           ps512.tile([128, 512], F32, name="ps_p1", tag="a")]
                    for kt in range(KT):
                        for ci, (c0, w) in enumerate(DCH):
                            nc.tensor.matmul(
                                pss[ci][:m, :w],
                                oT[:, kt, 128 * tt:128 * tt + m],
                                w_proj[:, kt, c0:c0 + w],
                                start=(kt == 0), stop=(kt == KT - 1),
                            )
                    for ci, (c0, w) in enumerate(DCH):
                        nc.vector.tensor_add(
                            out_sb[:m, c0:c0 + w], pss[ci][:m, :w],
                            pbias[:m, c0:c0 + w],
                        )
                    nc.sync.dma_start(
                        out=out_d[b, 128 * tt:128 * tt + m, :],
                        in_=out_sb[:m, :],
                    )
    nc.finalize()
    return nc


_NC_CACHE = {}


def _get_nc():
    if "nc" not in _NC_CACHE:
        _NC_CACHE["nc"] = _build_nc()
    return _NC_CACHE["nc"]


def _prep_shared(qkv_w, q_bias, v_bias, rpb_table, proj_w, proj_b, rel_index):
    qkv_w = np.asarray(qkv_w, dtype=np.float32).copy()
    qkv_w[:D] *= SCALE                      # fold softmax scale into q rows
    qkv_bias = np.concatenate([
        np.asarray(q_bias, np.float32) * SCALE,
        np.zeros(D, np.float32),
        np.asarray(v_bias, np.float32),
    ])
    # [128, KT, 3D]: w[p, kt, m] = qkv_w[m, kt*128+p]
    wqkv = np.ascontiguousarray(
        qkv_w.T.reshape(KT, 128, 3 * D).transpose(1, 0, 2)).astype(BF16)
    wproj = np.ascontiguousarray(
        np.asarray(proj_w, np.float32).T.reshape(KT, 128, D)
        .transpose(1, 0, 2)).astype(BF16)
    qkvb = np.ascontiguousarray(qkv_bias.reshape(18, 128).T).astype(np.float32)
    # relative position bias, transposed to [k, q] and padded to 640 rows
    rb = np.asarray(rpb_table, np.float32)[
        np.asarray(rel_index).reshape(-1)].reshape(N, N, NH)  # [q, k, h]
    rbp = np.zeros((TT * 128, N, NH), np.float32)
    rbp[:N] = rb.transpose(1, 0, 2)                            # [k, q, h]
    biasT = np.ascontiguousarray(
        rbp.reshape(TT, 128, N, NH).transpose(1, 3, 0, 2)).astype(BF16)
    vb = np.ascontiguousarray(qkv_bias[2 * D:].reshape(1, D)).astype(np.float32)
    pb = np.ascontiguousarray(np.asarray(proj_b, np.float32).reshape(1, D))
    return wqkv, wproj, qkvb, biasT, vb, pb


def kernel(**inputs):
    x = np.asarray(inputs["x"], dtype=np.float32)
    wqkv, wproj, qkvb, biasT, vb, pb = _prep_shared(
        inputs["qkv_w"], inputs["q_bias"], inputs["v_bias"],
        inputs["rpb_table"], inputs["proj_w"], inputs["proj_b"],
        inputs["rel_index"])

    in_maps = []
    for i in range(NCORES):
        xs = x[i * BL:(i + 1) * BL]                            # [BL, N, D]
        xT = np.ascontiguousarray(
            xs.transpose(0, 2, 1).reshape(BL, KT, 128, N)
            .transpose(0, 2, 1, 3)).astype(BF16)               # [BL,128,KT,N]
        in_maps.append({
            "xT": xT, "wqkv": wqkv, "wproj": wproj, "biasT": biasT,
            "qkvb": qkvb, "vb": vb, "pb": pb,
        })

    nc = _get_nc()
    res = run_bass_kernel_spmd(nc, in_maps, core_ids=list(range(NCORES)))
    out = np.concatenate([res.results[i]["out"] for i in range(NCORES)], axis=0)
    return np.ascontiguousarray(out.astype(np.float32))


# revision 15
# speedup vs baseline: 9979.0699x; 9979.0699x over previous
"""BEiT-style windowed attention (B=32, N=577, D=768, 12 heads) on 8 TRN2 cores.

Strategy: pure data-parallel over batch (4 batch elements per core, no
collectives). All matmuls in bf16 (fp32 PSUM accumulate), softmax in fp32.

Per-core dataflow (all activations kept SBUF-resident per batch element):
  qkT  [d,tok] = W_qkv(T) @ xT          (q/k produced transposed)
  v    [tok,d] = xT(T) @ W_qkvT          (v produced natural, +ones col)
  S.T  [k,q]   = kT(T) @ qT              (scores transposed, K=64)
  P    = exp(S.T + relbias.T)            (unsafe softmax: logits are tiny)
  O_un [q,d+1] = P(T) @ v_aug            (ones col gives rowsum per q)
  O    = O_un[:, :64] * recip(rowsum)    (per-partition scale)
  OT   = transpose(O)                    (PE transposes, 30 per batch)
  out  [tok,od] = OT(T) @ W_projT + b    (natural layout -> contiguous DMA)

Host-side prep: transposed/bf16 weights, x transposed to [d,tok] tiles,
relative-position bias table gathered + transposed, softmax scale folded
into the q rows of W_qkv.
"""

import numpy as np
import ml_dtypes

import concourse.bass as bass
import concourse.tile as tile
from concourse import bacc
from concourse import mybir
from concourse.bass_utils import run_bass_kernel_spmd
from concourse.masks import make_identity

B, N, D = 32, 577, 768
NH, DH = 12, 64
NCORES = 8
BL = B // NCORES            # 4 batch elements per core
SCALE = DH ** -0.5
KT = D // 128               # 6 contraction tiles over D
TT = (N + 127) // 128       # 5 token tiles (4x128 + 65)
BF16 = ml_dtypes.bfloat16

F32 = mybir.dt.float32
BF = mybir.dt.bfloat16


def tok_m(t):
    return min(128, N - 128 * t)


def _build_nc():
    nc = bacc.Bacc()

    xT_d = nc.declare_dram_parameter("xT", [BL, 128, KT, N], BF, isOutput=False)
    wqkv_d = nc.declare_dram_parameter("wqkv", [128, KT, 3 * D], BF, isOutput=False)
    wproj_d = nc.declare_dram_parameter("wproj", [128, KT, D], BF, isOutput=False)
    biasT_d = nc.declare_dram_parameter("biasT", [128, NH, TT, N], BF, isOutput=False)
    qkvb_d = nc.declare_dram_parameter("qkvb", [128, 18], F32, isOutput=False)
    vb_d = nc.declare_dram_parameter("vb", [1, D], F32, isOutput=False)
    pb_d = nc.declare_dram_parameter("pb", [1, D], F32, isOutput=False)
    out_d = nc.declare_dram_parameter("out", [BL, N, D], F32, isOutput=True)

    Exp = mybir.ActivationFunctionType.Exp
    QCH = [(0, 512), (512, N - 512)]          # free-dim chunks over 577
    DCH = [(0, 512), (512, D - 512)]          # free-dim chunks over 768

    with tile.TileContext(nc) as tc:
        with (
            tc.tile_pool(name="singles", bufs=1) as singles,
            tc.tile_pool(name="xt", bufs=2) as xt_pool,
            tc.tile_pool(name="qkt", bufs=2) as qkt_pool,
            tc.tile_pool(name="vbuf", bufs=1) as v_pool,
            tc.tile_pool(name="exps", bufs=10) as exps_pool,
            tc.tile_pool(name="praw", bufs=2) as praw_pool,
            tc.tile_pool(name="obuf", bufs=1) as o_pool,
            tc.tile_pool(name="otb", bufs=1) as ot_pool,
            tc.tile_pool(name="outs", bufs=2) as out_pool,
            tc.tile_pool(name="small", bufs=4) as small_pool,
            tc.tile_pool(name="ps512", bufs=2, space="PSUM") as ps512,
            tc.tile_pool(name="ps128", bufs=2, space="PSUM") as ps128,
            tc.tile_pool(name="psS", bufs=2, space="PSUM") as psS_pool,
        ):
            # ---- one-time loads (small first; big biasT on the second
            # HWDGE queue so it streams while qkv starts) ----
            qkvb = singles.tile([128, 18], F32)
            nc.sync.dma_start(out=qkvb, in_=qkvb_d[:])
            vbias = singles.tile([128, D], F32)
            nc.sync.dma_start(out=vbias, in_=vb_d[:].to_broadcast([128, D]))
            pbias = singles.tile([128, D], F32)
            nc.sync.dma_start(out=pbias, in_=pb_d[:].to_broadcast([128, D]))
            w_qkv = singles.tile([128, KT, 3 * D], BF)
            nc.sync.dma_start(out=w_qkv, in_=wqkv_d[:])
            w_proj = singles.tile([128, KT, D], BF)
            nc.sync.dma_start(out=w_proj, in_=wproj_d[:])
            biasT = singles.tile([128, NH, TT, N], BF)
            nc.scalar.dma_start(out=biasT, in_=biasT_d[:])
            ident = singles.tile([128, 128], BF)
            make_identity(nc, ident)

            for b in range(BL):
                xT = xt_pool.tile([128, KT, N], BF)
                nc.sync.dma_start(out=xT, in_=xT_d[b])

                # ---- qkv projection: q,k transposed [d, tok] ----
                qkT = qkt_pool.tile([128, 2 * KT, N], BF)
                for mt in range(2 * KT):
                    pss = [ps512.tile([128, 512], F32, name="ps_qk0", tag="a"),
                           ps128.tile([128, 128], F32, name="ps_qk1", tag="b")]
                    for kt in range(KT):
                        for ci, (c0, w) in enumerate(QCH):
                            nc.tensor.matmul(
                                pss[ci][:, :w],
                                w_qkv[:, kt, 128 * mt:128 * (mt + 1)],
                                xT[:, kt, c0:c0 + w],
                                start=(kt == 0), stop=(kt == KT - 1),
                            )
                    for ci, (c0, w) in enumerate(QCH):
                        nc.vector.tensor_add(
                            qkT[:, mt, c0:c0 + w], pss[ci][:, :w],
                            qkvb[:, mt:mt + 1].to_broadcast([128, w]),
                        )

                # ---- qkv projection: v natural [tok, d], strided + ones col ----
                v_sb = v_pool.tile([128, TT, NH * 65], BF)
                v_str = v_sb.rearrange("p t (h c) -> p t h c", c=65)
                nc.vector.memset(v_str[:, :, :, 64:65], 1.0)
                for tt in range(TT):
                    m = tok_m(tt)
                    pss = [ps512.tile([128, 512], F32, name="ps_v0", tag="a"),
                           ps512.tile([128, 512], F32, name="ps_v1", tag="a")]
                    for kt in range(KT):
                        for ci, (c0, w) in enumerate(DCH):
                            nc.tensor.matmul(
                                pss[ci][:m, :w],
                                xT[:, kt, 128 * tt:128 * tt + m],
                                w_qkv[:, kt, 2 * D + c0:2 * D + c0 + w],
                                start=(kt == 0), stop=(kt == KT - 1),
                            )
                    for ci, (c0, w) in enumerate(DCH):
                        nh0, nh1 = c0 // 64, (c0 + w) // 64
                        nc.vector.tensor_add(
                            v_str[:m, tt, nh0:nh1, 0:64],
                            pss[ci][:m, :w].rearrange("p (h c) -> p h c", c=64),
                            vbias[:m, c0:c0 + w].rearrange("p (h c) -> p h c", c=64),
                        )

                # ---- attention per head ----
                o_sb = o_pool.tile([128, TT, D], BF)
                for h in range(NH):
                    qT = qkT[64 * (h % 2):64 * (h % 2) + 64, h // 2, :]
                    kTh = qkT[64 * (h % 2):64 * (h % 2) + 64, KT + h // 2, :]
                    expS = [exps_pool.tile([128, N], BF, name="expS", tag="es")
                            for _ in range(TT)]
                    for kt in range(TT):
                        km = tok_m(kt)
                        ps_s = psS_pool.tile([128, N], F32, name="ps_s")
                        for ci, (c0, w) in enumerate(QCH):
                            nc.tensor.matmul(
                                ps_s[:km, c0:c0 + w],
                                kTh[:, 128 * kt:128 * kt + km],
                                qT[:, c0:c0 + w],
                                start=True, stop=True,
                            )
                        praw = praw_pool.tile([128, N], BF)
                        nc.scalar.activation(praw[:km, :], ps_s[:km, :], Exp)
                        # multiply in exp(rel_bias), precomputed on host;
                        # spread across DVE and the otherwise-idle GpSimd
                        eng = nc.gpsimd if kt % 3 == 2 else nc.vector
                        eng.tensor_mul(
                            expS[kt][:km, :], praw[:km, :],
                            biasT[:km, h, kt, :],
                        )
                    # P @ V_aug  (out natural [q, 64+1]; col 64 = rowsum)
                    for qt in range(TT):
                        qm = tok_m(qt)
                        ps_o = ps128.tile([128, 128], F32, name="ps_o", tag="b")
                        for kt in range(TT):
                            km = tok_m(kt)
                            nc.tensor.matmul(
                                ps_o[:qm, :65],
                                expS[kt][:km, 128 * qt:128 * qt + qm],
                                v_sb[:km, kt, 65 * h:65 * h + 65],
                                start=(kt == 0), stop=(kt == TT - 1),
                            )
                        rcp = small_pool.tile([128, 1], F32)
                        nc.vector.reciprocal(rcp[:qm], ps_o[:qm, 64:65])
                        nc.vector.tensor_mul(
                            o_sb[:qm, qt, 64 * h:64 * h + 64],
                            ps_o[:qm, 0:64],
                            rcp[:qm, 0:1].to_broadcast([qm, 64]),
                        )

                # ---- transpose O -> OT [d, tok] ----
                oT = ot_pool.tile([128, KT, N], BF)
                for qt in range(TT):
                    qm = tok_m(qt)
                    for dt in range(KT):
                        ps_t = ps128.tile([128, 128], BF, name="ps_t", tag="b")
                        nc.tensor.transpose(
                            ps_t[:, :qm],
                            o_sb[:qm, qt, 128 * dt:128 * (dt + 1)],
                            ident[:qm, :qm],
                        )
                        nc.scalar.activation(
                            oT[:, dt, 128 * qt:128 * qt + qm], ps_t[:, :qm],
                            mybir.ActivationFunctionType.Copy,
                        )

                # ---- output projection ----
                for tt in range(TT):
                    m = tok_m(tt)
                    out_sb = out_pool.tile([128, D], F32)
                    pss = [ps512.tile([128, 512], F32, name="ps_p0", tag="a"),
                           ps512.tile([128, 512], F32, name="ps_p1", tag="a")]
                    for kt in range(KT):
                        for ci, (c0, w) in enumerate(DCH):
                            nc.tensor.matmul(
                                pss[ci][:m, :w],
                                oT[:, kt, 128 * tt:128 * tt + m],
                                w_proj[:, kt, c0:c0 + w],
                                start=(kt == 0), stop=(kt == KT - 1),
                            )
                    for ci, (c0, w) in enumerate(DCH):
                        nc.vector.tensor_add(
                            out_sb[:m, c0:c0 + w], pss[ci][:m, :w],
                            pbias[:m, c0:c0 + w],
                        )
                    nc.sync.dma_start(
                        out=out_d[b, 128 * tt:128 * tt + m, :],
                        in_=out_sb[:m, :],
                    )
    nc.finalize()
    return nc


_NC_CACHE = {}


def _get_nc():
    if "nc" not in _NC_CACHE:
        _NC_CACHE["nc"] = _build_nc()
    return _NC_CACHE["nc"]


def _prep_shared(qkv_w, q_bias, v_bias, rpb_table, proj_w, proj_b, rel_index):
    qkv_w = np.asarray(qkv_w, dtype=np.float32).copy()
    qkv_w[:D] *= SCALE                      # fold softmax scale into q rows
    qkv_bias = np.concatenate([
        np.asarray(q_bias, np.float32) * SCALE,
        np.zeros(D, np.float32),
        np.asarray(v_bias, np.float32),
    ])
    # [128, KT, 3D]: w[p, kt, m] = qkv_w[m, kt*128+p]
    wqkv = np.ascontiguousarray(
        qkv_w.T.reshape(KT, 128, 3 * D).transpose(1, 0, 2)).astype(BF16)
    wproj = np.ascontiguousarray(
        np.asarray(proj_w, np.float32).T.reshape(KT, 128, D)
        .transpose(1, 0, 2)).astype(BF16)
    qkvb = np.ascontiguousarray(qkv_bias.reshape(18, 128).T).astype(np.float32)
    # relative position bias, transposed to [k, q] and padded to 640 rows
    rb = np.asarray(rpb_table, np.float32)[
        np.asarray(rel_index).reshape(-1)].reshape(N, N, NH)  # [q, k, h]
    rbp = np.zeros((TT * 128, N, NH), np.float32)
    rbp[:N] = rb.transpose(1, 0, 2)                            # [k, q, h]
    biasT = np.ascontiguousarray(
        np.exp(rbp.reshape(TT, 128, N, NH).transpose(1, 3, 0, 2))).astype(BF16)
    vb = np.ascontiguousarray(qkv_bias[2 * D:].reshape(1, D)).astype(np.float32)
    pb = np.ascontiguousarray(np.asarray(proj_b, np.float32).reshape(1, D))
    return wqkv, wproj, qkvb, biasT, vb, pb


def _make_in_maps(inputs):
    x = np.asarray(inputs["x"], dtype=np.float32)
    wqkv, wproj, qkvb, biasT, vb, pb = _prep_shared(
        inputs["qkv_w"], inputs["q_bias"], inputs["v_bias"],
        inputs["rpb_table"], inputs["proj_w"], inputs["proj_b"],
        inputs["rel_index"])

    in_maps = []
    for i in range(NCORES):
        xs = x[i * BL:(i + 1) * BL]                            # [BL, N, D]
        xT = np.ascontiguousarray(
            xs.transpose(0, 2, 1).reshape(BL, KT, 128, N)
            .transpose(0, 2, 1, 3)).astype(BF16)               # [BL,128,KT,N]
        in_maps.append({
            "xT": xT, "wqkv": wqkv, "wproj": wproj, "biasT": biasT,
            "qkvb": qkvb, "vb": vb, "pb": pb,
        })

    return in_maps


def kernel(**inputs):
    in_maps = _make_in_maps(inputs)
    nc = _get_nc()
    res = run_bass_kernel_spmd(nc, in_maps, core_ids=list(range(NCORES)))
    out = np.concatenate([res.results[i]["out"] for i in range(NCORES)], axis=0)
    return np.ascontiguousarray(out.astype(np.float32))


def kernel_traced(**inputs):
    """Like kernel() but also returns (out, BassKernelResults with profile)."""
    in_maps = _make_in_maps(inputs)
    nc = _get_nc()
    res = run_bass_kernel_spmd(nc, in_maps, core_ids=list(range(NCORES)),
                               trace=True)
    out = np.concatenate([res.results[i]["out"] for i in range(NCORES)], axis=0)
    return np.ascontiguousarray(out.astype(np.float32)), res
